# revision 5
# baseline (speedup 1.0000x reference)
"""Bag-attention (NRE selective attention) kernel for 8 TRN2 NeuronCores, v4.

Reference computation:
    logit_i = sum_d x[i,d] * aw[q_i,d] * rw[q_i,d]
    w       = segment_softmax(logit, seg)        (bags = contiguous ranges)
    bag[b]  = sum_{i in b} w_i * x[i]
    out     = bag @ rw.T + bias

Split: the device computes the only O(N*D*C) term, the per-sentence
projection P_i = x_i @ rw.T (f32 PSUM accumulation, bf16 output). The
host computes the O(N*D) attention logits exactly from f32 x, then
finishes with the O(N*C) ragged segment softmax / reduceat:
    out[b] = reduceat(e*P) / reduceat(e) + bias,  e = exp(logit)
(exp without max-subtraction is safe: logits have std ~0.1).

Quantization scheme (validated vs the reference in fp64, rel ~6.8e-3):
  - x ships as fp8 e4m3 (1 B/elem, 11.3 MB/core vs 18.4 baseline).
  - Per-sentence fp8 noise is averaged down by the softmax inside large
    bags but survives in small bags, so the ~16K sentences in the
    smallest bags (global priority order, capped at 2048/core) also ship
    an e5m2 *residual* x - e4m3(x); one extra 2048-col matmul block per
    core produces bf16 P-corrections the host adds back by index.
  - Weight quantization error is systematic per bag (doesn't average
    out), so W is applied in two fp8 passes accumulating in PSUM:
    e4m3(W) + e5m2(W - e4m3(W)) ~= bf16-accurate W.
  - fp8 x * fp8 W enables MatmulPerfMode.DoubleRow (paired 128-deep
    k-tiles, 0.5 cyc/row) for dims 0:512. Dims 512:690 (chunk of 128 +
    tail of 50, unpairable without zero-padding DMA) use single-pass
    bf16 stationaries at 1 cyc/row; PE stays under the DMA floor.

Per-core traffic: 12.8 MB in + 2.1 MB out ~= 14.9 MB vs ~358 GB/s HBM.

Sharding: 16384 contiguous sentences per core; weights replicated; the
ragged segment softmax runs on the host so bags straddling core
boundaries need no special handling. The correction block is decoupled
from sentence ownership (any core may correct any sentence), so the 8
correction blocks are filled round-robin and perfectly balanced.
"""

import sys

_REPO = "/opt/trn_rl_repo"
if _REPO not in sys.path:
    sys.path.insert(0, _REPO)

import numpy as np
import ml_dtypes

N_SENT = 131072
REL_DIM = 690
C = 53
NCORES = 8
NS = N_SENT // NCORES  # sentences per core
PMAIN = 128
NCHM = 5               # main chunks of 128 dims (0:640)
TAIL = 50              # tail dims 640:690
NDR = 2                # DoubleRow pairs = chunks 0:4 (dims 0:512)
BLK = 2048             # sentences per compute block
SUB = 512              # matmul col slice (one PSUM bank = 512 f32)
NBLK = NS // BLK       # 8 main blocks; block NBLK is the correction block
MM = 64                # matmul stationary cols / PSUM partitions (53 + 11
                       # zero pad; the dual-fp8 Ldweights encoding rejects 53)
OROWS = 56             # output rows (53 + 3 pad; 56 -> 14 DMA engines)
CORR_CAP = NCORES * BLK

_NC_CACHE = {}


def _build():
    import concourse.bass as bass
    from concourse import mybir

    f32 = mybir.dt.float32
    bf16 = mybir.dt.bfloat16
    e4 = mybir.dt.float8e4
    e5 = mybir.dt.float8e5
    DR = mybir.MatmulPerfMode.DoubleRow

    nc = bass.Bass()
    xm = nc.declare_dram_parameter("xm", [PMAIN, NBLK, NCHM, BLK], e4, isOutput=False)
    xt = nc.declare_dram_parameter("xt", [TAIL, NBLK, BLK], e4, isOutput=False)
    cm = nc.declare_dram_parameter("cm", [PMAIN, NCHM, BLK], e5, isOutput=False)
    ct = nc.declare_dram_parameter("ct", [TAIL, BLK], e5, isOutput=False)
    w8 = nc.declare_dram_parameter("w8", [PMAIN, NDR * 2 * MM], e4, isOutput=False)
    wr = nc.declare_dram_parameter("wr", [PMAIN, NDR * 2 * MM], e5, isOutput=False)
    wb = nc.declare_dram_parameter("wb", [PMAIN, 2 * MM], bf16, isOutput=False)
    out = nc.declare_dram_parameter("out", [OROWS, (NBLK + 1) * BLK], bf16,
                                    isOutput=True)

    from contextlib import ExitStack
    with ExitStack() as stk:
        xbuf = stk.enter_context(nc.sbuf_tensor("xbuf", [PMAIN, NBLK, NCHM, BLK], e4))
        xtail = stk.enter_context(nc.sbuf_tensor("xtail", [TAIL, NBLK, BLK], e4))
        cbuf = stk.enter_context(nc.sbuf_tensor("cbuf", [PMAIN, NCHM, BLK], e5))
        ctail = stk.enter_context(nc.sbuf_tensor("ctail", [TAIL, BLK], e5))
        w8_sb = stk.enter_context(nc.sbuf_tensor("w8_sb", [PMAIN, NDR, 2, MM], e4))
        wr_sb = stk.enter_context(nc.sbuf_tensor("wr_sb", [PMAIN, NDR, 2, MM], e5))
        wb_sb = stk.enter_context(nc.sbuf_tensor("wb_sb", [PMAIN, 2 * MM], bf16))
        out_sb = stk.enter_context(nc.sbuf_tensor("out_sb", [OROWS, 2, BLK], bf16))
        psb = [stk.enter_context(nc.psum_tensor(f"ps{i}", [MM, BLK], f32))
               for i in range(2)]

        s_x = [stk.enter_context(nc.semaphore(f"s_x{i}")) for i in range(NBLK)]
        s_xt = stk.enter_context(nc.semaphore("s_xt"))
        s_c = stk.enter_context(nc.semaphore("s_c"))
        s_w = stk.enter_context(nc.semaphore("s_w"))
        s_mm = stk.enter_context(nc.semaphore("s_mm"))
        s_cp = stk.enter_context(nc.semaphore("s_cp"))
        s_od = stk.enter_context(nc.semaphore("s_od"))
        block = stk.enter_context(nc.Block())

        @block.sync
        def _(sync):
            for db in range(NBLK):
                sync.dma_start(
                    out=xbuf[:, db, :, :], in_=xm[:, db, :, :],
                ).then_inc(s_x[db], 16)

        @block.gpsimd
        def _(gp):
            for db in range(NBLK):
                # just-in-time: don't steal stream bandwidth from earlier
                # main dblocks (tail db is only needed alongside main db)
                if db >= 1:
                    gp.wait_ge(s_x[db - 1], 16)
                gp.dma_start(
                    out=xtail[:, db, :], in_=xt[:, db, :],
                ).then_inc(s_xt, 16)
            gp.dma_start(out=cbuf[:], in_=cm[:]).then_inc(s_c, 16)
            gp.dma_start(out=ctail[:], in_=ct[:]).then_inc(s_c, 16)

        @block.tensor
        def _(pe):
            pe.wait_ge(s_w, 48)
            for b in range(NBLK + 1):
                i = b % 2
                corr = b == NBLK
                if corr:
                    pe.wait_ge(s_c, 32)
                else:
                    pe.wait_ge(s_x[b], 16)
                    pe.wait_ge(s_xt, 16 * (b + 1))
                if b >= 2:
                    pe.wait_ge(s_cp, b - 1)  # copy(b-2) freed ps[i]
                for sub in range(BLK // SUB):
                    s0, s1 = sub * SUB, (sub + 1) * SUB
                    ps = psb[i][:, s0:s1]
                    if corr:
                        mv = [cbuf[:, 0:2, s0:s1], cbuf[:, 2:4, s0:s1],
                              cbuf[:, 4, s0:s1], ctail[:, s0:s1]]
                    else:
                        mv = [xbuf[:, b, 0:2, s0:s1], xbuf[:, b, 2:4, s0:s1],
                              xbuf[:, b, 4, s0:s1], xtail[:, b, s0:s1]]
                    for p in range(NDR):
                        nc.tensor.matmul(
                            ps, w8_sb[:, p, :, :], mv[p],
                            start=(p == 0), stop=False, perf_mode=DR,
                        )
                        nc.tensor.matmul(
                            ps, wr_sb[:, p, :, :], mv[p],
                            start=False, stop=False, perf_mode=DR,
                        )
                    nc.tensor.matmul(
                        ps, wb_sb[:, 0:MM], mv[2],
                        start=False, stop=False,
                    )
                    mmt = nc.tensor.matmul(
                        ps, wb_sb[0:TAIL, MM:2 * MM], mv[3],
                        start=False, stop=True,
                    )
                mmt.then_inc(s_mm, 1)

        @block.vector
        def _(dve):
            nc.vector.memset(out_sb[:, :, :], 0.0)
            for b in range(NBLK + 1):
                i = b % 2
                dve.wait_ge(s_mm, b + 1)
                if b >= 2:
                    dve.wait_ge(s_od, 16 * (b - 1))  # out-DMA(b-2) freed out_sb[i]
                nc.vector.tensor_copy(
                    out_sb[0:C, i, :], psb[i][0:C, :]
                ).then_inc(s_cp, 1)

        @block.scalar
        def _(act):
            nc.scalar.dma_start(out=w8_sb[:], in_=w8[:]).then_inc(s_w, 16)
            nc.scalar.dma_start(out=wr_sb[:], in_=wr[:]).then_inc(s_w, 16)
            nc.scalar.dma_start(out=wb_sb[:], in_=wb[:]).then_inc(s_w, 16)
            for b in range(NBLK + 1):
                i = b % 2
                act.wait_ge(s_cp, b + 1)
                nc.scalar.dma_start(
                    out=out[:, b * BLK:(b + 1) * BLK],
                    in_=out_sb[:, i, :],
                ).then_inc(s_od, 16)

    return nc


def _get_nc():
    if "nc" not in _NC_CACHE:
        _NC_CACHE["nc"] = _build()
    return _NC_CACHE["nc"]


def _pack_x(xq8, ns_rows):
    """[ns, 690] fp8 -> main [128, NBLK?, 5, blk] + tail [50, ...]."""
    nblk = ns_rows // BLK
    t = np.ascontiguousarray(xq8.T)  # [690, ns]
    main = t[0:PMAIN * NCHM].reshape(NCHM, PMAIN, nblk, BLK).transpose(1, 2, 0, 3)
    tail = t[PMAIN * NCHM:].reshape(TAIL, nblk, BLK)
    return np.ascontiguousarray(main), np.ascontiguousarray(tail)


def _prepare(x, relation_weight, scope):
    e4m3 = ml_dtypes.float8_e4m3
    e5m2 = ml_dtypes.float8_e5m2
    bf16 = ml_dtypes.bfloat16
    x = np.asarray(x, dtype=np.float32)
    rw = np.asarray(relation_weight, dtype=np.float32)

    wmat = np.zeros((REL_DIM, MM), dtype=np.float32)
    wmat[:, 0:C] = rw.T  # zero-pad cols 53:64 (dual-fp8 Ldweights needs 64)
    w8f = wmat[0:2 * 256].astype(e4m3)
    wrf = (wmat[0:2 * 256] - w8f.astype(np.float32)).astype(e5m2)
    # [512, 64] -> [128, NDR, 2, 64]: row r of pair p, k-tile k = dim
    # p*256 + k*128 + r
    w8_p = np.ascontiguousarray(
        w8f.reshape(NDR, 2, PMAIN, MM).transpose(2, 0, 1, 3)).reshape(PMAIN, -1)
    wr_p = np.ascontiguousarray(
        wrf.reshape(NDR, 2, PMAIN, MM).transpose(2, 0, 1, 3)).reshape(PMAIN, -1)
    wb_p = np.zeros((PMAIN, 2 * MM), dtype=bf16)
    wb_p[:, 0:MM] = wmat[512:640].astype(bf16)
    wb_p[0:TAIL, MM:2 * MM] = wmat[640:690].astype(bf16)

    x8 = x.astype(e4m3)

    # correction set: sentences in the globally smallest bags, capped
    scope = np.asarray(scope).astype(np.int64)
    sizes = np.diff(scope)
    seg = np.repeat(np.arange(sizes.shape[0]), sizes)
    ssz = sizes[seg]                          # bag size per sentence
    order = np.argsort(ssz, kind="stable")
    ncorr = int(min(CORR_CAP, int((ssz <= 16).sum())))
    corr_idx = order[:ncorr]

    r_all = np.zeros((CORR_CAP, REL_DIM), dtype=e5m2)
    r_all[:ncorr] = (x[corr_idx] - x8[corr_idx].astype(np.float32)).astype(e5m2)

    in_maps = []
    for m in range(NCORES):
        sl = slice(m * NS, (m + 1) * NS)
        xm_p, xt_p = _pack_x(x8[sl], NS)
        cm_p, ct_p = _pack_x(r_all[m * BLK:(m + 1) * BLK], BLK)
        in_maps.append({
            "xm": xm_p, "xt": xt_p,
            "cm": np.ascontiguousarray(cm_p[:, 0]),
            "ct": np.ascontiguousarray(ct_p[:, 0]),
            "w8": w8_p, "wr": wr_p, "wb": wb_p,
        })
    return in_maps, corr_idx


def _finish(P, x, aw, rw, attention_query, scope, bias):
    m = (aw * rw).astype(np.float32)  # [53, 690]
    q = np.asarray(attention_query).astype(np.int64)
    logit = np.empty(N_SENT, dtype=np.float32)
    step = 16384
    for i in range(0, N_SENT, step):
        logit[i:i + step] = np.einsum(
            "nd,nd->n", x[i:i + step], m[q[i:i + step]], optimize=True)
    e = np.exp(logit.astype(np.float64))
    scope = np.asarray(scope).astype(np.int64)
    sums = np.add.reduceat(P * e[:, None], scope[:-1], axis=0)
    esum = np.add.reduceat(e, scope[:-1])
    logits = sums / esum[:, None] + np.asarray(bias, np.float64)[None, :]
    return logits.astype(np.float32)


def _run(inputs, trace=False, **kw):
    from concourse.bass_utils import run_bass_kernel_spmd

    nc = _get_nc()
    x = np.asarray(inputs["x"], dtype=np.float32)
    in_maps, corr_idx = _prepare(x, inputs["relation_weight"], inputs["scope"])
    res = run_bass_kernel_spmd(nc, in_maps, core_ids=list(range(NCORES)),
                               trace=trace, **kw)
    outs = np.stack([np.asarray(r["out"]) for r in res.results])
    GP = np.asarray(outs, dtype=np.float32)
    P = GP[:, 0:C, 0:NS].transpose(0, 2, 1).reshape(N_SENT, C).astype(np.float64)
    Pc = GP[:, 0:C, NS:].transpose(0, 2, 1).reshape(CORR_CAP, C)
    ncorr = corr_idx.shape[0]
    P[corr_idx] += Pc[:ncorr].astype(np.float64)
    logits = _finish(
        P, x,
        np.asarray(inputs["attention_weight"], dtype=np.float32),
        np.asarray(inputs["relation_weight"], dtype=np.float32),
        inputs["attention_query"], inputs["scope"],
        np.asarray(inputs["bias"], np.float32))
    return logits, res


def kernel(x, relation_weight, attention_weight, bias, attention_query, scope):
    logits, _ = _run(dict(x=x, relation_weight=relation_weight,
                          attention_weight=attention_weight, bias=bias,
                          attention_query=attention_query, scope=scope))
    return logits


# revision 8
# speedup vs baseline: 1.4301x; 1.4301x over previous
"""Bag-attention (NRE selective attention) kernel for 8 TRN2 NeuronCores, v5.

Reference computation:
    logit_i = sum_d x[i,d] * aw[q_i,d] * rw[q_i,d]
    w       = segment_softmax(logit, seg)        (bags = contiguous ranges)
    bag[b]  = sum_{i in b} w_i * x[i]
    out     = bag @ rw.T + bias

Split: the device computes the O(N*D*C) per-sentence projection
P_i = x_i @ W8 (f32 PSUM, bf16 out) over fp8-e4m3 inputs. The host
computes the O(N*D) attention logits exactly from f32 x, applies two
exact low-cost fixes, and finishes the O(N*C) ragged segment softmax:
    out[b] = reduceat(e*P)/reduceat(e) + (A_b/esum_b) @ dW + bias
where dW = W - e4m3(W) (the systematic weight-quantization residual,
A_b = reduceat(e * x_hat) - an O(N*D) reduction + [B,D]@[D,C] matmul),
and e = exp(logit) (safe without max-subtraction: logit std ~0.1).

Measured HW facts this design is built on (probed on these cores):
  - Every Matmult costs ~out_cols * 0.417ns + ~68ns fixed, regardless of
    dtype; a matmul output cannot span PSUM banks (=512 f32 cols), so
    the only PE lever is FEWER instructions.
  - MatmulPerfMode.DoubleRow (both operands fp8-e4/e5) contracts TWO
    128-row k-tiles per instruction at the same cost -> 690 dims = 3 DR
    matmuls per 512-col slice (pairs (0,1),(2,3),(4,tail)); stationary
    cols must be 64 (53 rejects the dual-fp8 Ldweights encoding).
  - 8 queued input DMAs progress in parallel (packet round-robin), so
    an unpaced stream completes block 0 last; depth-2 pacing on the
    issuing sequencer restores FIFO-ish completion.

Quantization (validated vs reference in f64, rel ~6.7e-3 < 2e-2):
  - x ships as e4m3 (11.3 MB/core vs 18.4 baseline); per-sentence fp8
    noise averages out inside large bags, and the ~16K sentences in the
    globally smallest bags ship an extra e5m2 residual x - e4m3(x)
    whose bf16 P-corrections the host adds back by index (one extra
    2048-col block per core, round-robin balanced across cores).
  - W ships as a single e4m3 pass; its systematic error is removed by
    the exact host dW term above.

Per-core traffic ~14.9 MB vs ~330-358 GB/s HBM -> ~45 us floor; PE ~30 us.

Sharding: 16384 contiguous sentences per core; weights replicated; the
ragged segment softmax runs on the host so bags straddling core
boundaries need no special handling.
"""

import sys

_REPO = "/opt/trn_rl_repo"
if _REPO not in sys.path:
    sys.path.insert(0, _REPO)

import numpy as np
import ml_dtypes

N_SENT = 131072
REL_DIM = 690
C = 53
NCORES = 8
NS = N_SENT // NCORES  # sentences per core
PMAIN = 128
NCHM = 5               # main chunks of 128 dims (0:640)
TAIL = 50              # tail dims 640:690
NPAIR = 3              # DR pairs: (0,1), (2,3), (4, tail zero-padded)
BLK = 2048             # sentences per compute block
SUB = 512              # matmul col slice (one PSUM bank = 512 f32)
NBLK = NS // BLK       # 8 main blocks; block NBLK is the correction block
MM = 64                # stationary cols / PSUM partitions (53 + 11 zero
                       # pad; the dual-fp8 Ldweights encoding rejects 53)
OROWS = 56             # output rows (53 + 3 pad; 56 -> 14 DMA engines)
CORR_CAP = NCORES * BLK

_NC_CACHE = {}


def _build():
    import concourse.bass as bass
    from concourse import mybir

    f32 = mybir.dt.float32
    bf16 = mybir.dt.bfloat16
    e4 = mybir.dt.float8e4
    e5 = mybir.dt.float8e5
    DR = mybir.MatmulPerfMode.DoubleRow

    nc = bass.Bass()
    xm = nc.declare_dram_parameter("xm", [PMAIN, NBLK, NCHM, BLK], e4, isOutput=False)
    xt = nc.declare_dram_parameter("xt", [TAIL, NBLK, BLK], e4, isOutput=False)
    cm = nc.declare_dram_parameter("cm", [PMAIN, NCHM, BLK], e5, isOutput=False)
    ct = nc.declare_dram_parameter("ct", [TAIL, BLK], e5, isOutput=False)
    w8 = nc.declare_dram_parameter("w8", [PMAIN, NPAIR * 2 * MM], e4, isOutput=False)
    out = nc.declare_dram_parameter("out", [OROWS, (NBLK + 1) * BLK], bf16,
                                    isOutput=True)

    from contextlib import ExitStack
    with ExitStack() as stk:
        # 6 chunks of 128 dims; chunk 5 = dims 640:690 in rows 0:50, rows
        # 50:128 zero (memset once - paired with zero W rows, but fp8 NaN
        # bytes x 0 = NaN, so they must hold finite values)
        xbuf = stk.enter_context(nc.sbuf_tensor("xbuf", [PMAIN, NBLK, 6, BLK], e4))
        cbuf = stk.enter_context(nc.sbuf_tensor("cbuf", [PMAIN, 6, BLK], e5))
        w8_sb = stk.enter_context(nc.sbuf_tensor("w8_sb", [PMAIN, NPAIR, 2, MM], e4))
        out_sb = stk.enter_context(nc.sbuf_tensor("out_sb", [OROWS, 2, BLK], bf16))
        psb = [stk.enter_context(nc.psum_tensor(f"ps{i}", [MM, BLK], f32))
               for i in range(2)]

        s_x = [stk.enter_context(nc.semaphore(f"s_x{i}")) for i in range(NBLK)]
        s_xt = stk.enter_context(nc.semaphore("s_xt"))
        s_c = stk.enter_context(nc.semaphore("s_c"))
        s_w = stk.enter_context(nc.semaphore("s_w"))
        s_z = stk.enter_context(nc.semaphore("s_z"))
        s_mm = stk.enter_context(nc.semaphore("s_mm"))
        s_cp = stk.enter_context(nc.semaphore("s_cp"))
        s_od = stk.enter_context(nc.semaphore("s_od"))
        block = stk.enter_context(nc.Block())

        @block.sync
        def _(sync):
            for db in range(NBLK):
                # depth-2 pacing: completion order ~ issue order, so block
                # 0 lands early instead of all 8 finishing together
                if db >= 2:
                    sync.wait_ge(s_x[db - 2], 16)
                sync.dma_start(
                    out=xbuf[:, db, 0:NCHM, :], in_=xm[:, db, :, :],
                ).then_inc(s_x[db], 16)

        @block.gpsimd
        def _(gp):
            for db in range(NBLK):
                # memset(chunk5 region) must land before the 50-row tail
                # DMA partially overwrites it
                gp.wait_ge(s_z, 1 if db == 0 else 2)
                if db >= 1:
                    gp.wait_ge(s_x[db - 1], 16)
                gp.dma_start(
                    out=xbuf[0:TAIL, db, NCHM, :], in_=xt[:, db, :],
                ).then_inc(s_xt, 16)
            gp.dma_start(out=cbuf[:, 0:NCHM, :], in_=cm[:]).then_inc(s_c, 16)
            gp.wait_ge(s_z, 3)
            gp.dma_start(out=cbuf[0:TAIL, NCHM, :], in_=ct[:]).then_inc(s_c, 16)

        @block.tensor
        def _(pe):
            pe.wait_ge(s_w, 16)
            for b in range(NBLK + 1):
                i = b % 2
                corr = b == NBLK
                if corr:
                    pe.wait_ge(s_c, 32)
                else:
                    pe.wait_ge(s_x[b], 16)
                    pe.wait_ge(s_xt, 16 * (b + 1))
                if b >= 2:
                    pe.wait_ge(s_cp, b - 1)  # copy(b-2) freed ps[i]
                for sub in range(BLK // SUB):
                    s0, s1 = sub * SUB, (sub + 1) * SUB
                    ps = psb[i][:, s0:s1]
                    for p in range(NPAIR):
                        if corr:
                            mv = cbuf[:, 2 * p:2 * p + 2, s0:s1]
                        else:
                            mv = xbuf[:, b, 2 * p:2 * p + 2, s0:s1]
                        mmt = nc.tensor.matmul(
                            ps, w8_sb[:, p, :, :], mv,
                            start=(p == 0), stop=(p == NPAIR - 1),
                            perf_mode=DR,
                        )
                mmt.then_inc(s_mm, 1)

        @block.vector
        def _(dve):
            nc.vector.memset(xbuf[:, 0, NCHM, :], 0.0).then_inc(s_z, 1)
            nc.vector.memset(xbuf[:, 1:NBLK, NCHM, :], 0.0).then_inc(s_z, 1)
            nc.vector.memset(cbuf[:, NCHM, :], 0.0).then_inc(s_z, 1)
            nc.vector.memset(out_sb[:, :, :], 0.0)
            for b in range(NBLK + 1):
                i = b % 2
                dve.wait_ge(s_mm, b + 1)
                if b >= 2:
                    dve.wait_ge(s_od, 16 * (b - 1))  # out-DMA(b-2) freed out_sb[i]
                nc.vector.tensor_copy(
                    out_sb[0:C, i, :], psb[i][0:C, :]
                ).then_inc(s_cp, 1)

        @block.scalar
        def _(act):
            nc.scalar.dma_start(out=w8_sb[:], in_=w8[:]).then_inc(s_w, 16)
            for b in range(NBLK + 1):
                i = b % 2
                act.wait_ge(s_cp, b + 1)
                nc.scalar.dma_start(
                    out=out[:, b * BLK:(b + 1) * BLK],
                    in_=out_sb[:, i, :],
                ).then_inc(s_od, 16)

    return nc


def _get_nc():
    if "nc" not in _NC_CACHE:
        _NC_CACHE["nc"] = _build()
    return _NC_CACHE["nc"]


def _pack_x(xq8, ns_rows):
    """[ns, 690] fp8 -> main [128, nblk, 5, blk] + tail [50, nblk, blk]."""
    nblk = ns_rows // BLK
    t = np.ascontiguousarray(xq8.T)  # [690, ns]
    main = t[0:PMAIN * NCHM].reshape(NCHM, PMAIN, nblk, BLK).transpose(1, 2, 0, 3)
    tail = t[PMAIN * NCHM:].reshape(TAIL, nblk, BLK)
    return np.ascontiguousarray(main), np.ascontiguousarray(tail)


def _prepare(x, relation_weight, scope):
    e4m3 = ml_dtypes.float8_e4m3
    e5m2 = ml_dtypes.float8_e5m2
    x = np.asarray(x, dtype=np.float32)
    rw = np.asarray(relation_weight, dtype=np.float32)

    wmat = np.zeros((PMAIN * 6, MM), dtype=np.float32)
    wmat[0:REL_DIM, 0:C] = rw.T  # zero pad: cols 53:64, dim rows 690:768
    w8f = wmat.astype(e4m3)
    dW = (wmat - w8f.astype(np.float32))[0:REL_DIM, 0:C]  # [690, 53]
    # [768, 64] -> [128, pair, 2, 64]: row r, pair p, k-tile k = dim
    # p*256 + k*128 + r
    w8_p = np.ascontiguousarray(
        w8f.reshape(NPAIR, 2, PMAIN, MM).transpose(2, 0, 1, 3)).reshape(PMAIN, -1)

    x8 = x.astype(e4m3)

    # correction set: sentences in the globally smallest bags, capped
    scope = np.asarray(scope).astype(np.int64)
    sizes = np.diff(scope)
    seg = np.repeat(np.arange(sizes.shape[0]), sizes)
    ssz = sizes[seg]                          # bag size per sentence
    order = np.argsort(ssz, kind="stable")
    ncorr = int(min(CORR_CAP, int((ssz <= 16).sum())))
    corr_idx = order[:ncorr]

    r_all = np.zeros((CORR_CAP, REL_DIM), dtype=e5m2)
    r_all[:ncorr] = (x[corr_idx] - x8[corr_idx].astype(np.float32)).astype(e5m2)

    in_maps = []
    for m in range(NCORES):
        sl = slice(m * NS, (m + 1) * NS)
        xm_p, xt_p = _pack_x(x8[sl], NS)
        cm_p, ct_p = _pack_x(r_all[m * BLK:(m + 1) * BLK], BLK)
        in_maps.append({
            "xm": xm_p, "xt": xt_p,
            "cm": np.ascontiguousarray(cm_p[:, 0]),
            "ct": np.ascontiguousarray(ct_p[:, 0]),
            "w8": w8_p,
        })
    return in_maps, corr_idx, x8, r_all, dW


def _finish(P, x, x8, r_all, corr_idx, dW, aw, rw, attention_query, scope,
            bias):
    m = (aw * rw).astype(np.float32)  # [53, 690]
    q = np.asarray(attention_query).astype(np.int64)
    scope = np.asarray(scope).astype(np.int64)
    logit = np.empty(N_SENT, dtype=np.float32)
    step = 16384
    for i in range(0, N_SENT, step):
        logit[i:i + step] = np.einsum(
            "nd,nd->n", x[i:i + step], m[q[i:i + step]], optimize=True)
    e = np.exp(logit.astype(np.float64))
    esum = np.add.reduceat(e, scope[:-1])

    # exact fix of the systematic W-quantization term: (sum_bag e*x_hat)@dW
    ncorr = corr_idx.shape[0]
    xh = x8.astype(np.float32)
    xh[corr_idx[:ncorr]] += r_all[:ncorr].astype(np.float32)
    A = np.empty((scope.shape[0] - 1, REL_DIM), dtype=np.float64)
    ew = e[:, None]
    # chunked weighted reduceat to bound temp memory
    bstart = scope[:-1]
    A[:] = np.add.reduceat(xh * ew.astype(np.float32), bstart, axis=0)
    fix = (A / esum[:, None]) @ dW.astype(np.float64)

    sums = np.add.reduceat(P * e[:, None], scope[:-1], axis=0)
    logits = sums / esum[:, None] + fix + np.asarray(bias, np.float64)[None, :]
    return logits.astype(np.float32)


def _run(inputs, trace=False, **kw):
    from concourse.bass_utils import run_bass_kernel_spmd

    nc = _get_nc()
    x = np.asarray(inputs["x"], dtype=np.float32)
    in_maps, corr_idx, x8, r_all, dW = _prepare(
        x, inputs["relation_weight"], inputs["scope"])
    res = run_bass_kernel_spmd(nc, in_maps, core_ids=list(range(NCORES)),
                               trace=trace, **kw)
    outs = np.stack([np.asarray(r["out"]) for r in res.results])
    GP = np.asarray(outs, dtype=np.float32)
    P = GP[:, 0:C, 0:NS].transpose(0, 2, 1).reshape(N_SENT, C).astype(np.float64)
    Pc = GP[:, 0:C, NS:].transpose(0, 2, 1).reshape(CORR_CAP, C)
    ncorr = corr_idx.shape[0]
    P[corr_idx] += Pc[:ncorr].astype(np.float64)
    logits = _finish(
        P, x, x8, r_all, corr_idx, dW,
        np.asarray(inputs["attention_weight"], dtype=np.float32),
        np.asarray(inputs["relation_weight"], dtype=np.float32),
        inputs["attention_query"], inputs["scope"],
        np.asarray(inputs["bias"], np.float32))
    return logits, res


def kernel(x, relation_weight, attention_weight, bias, attention_query, scope):
    logits, _ = _run(dict(x=x, relation_weight=relation_weight,
                          attention_weight=attention_weight, bias=bias,
                          attention_query=attention_query, scope=scope))
    return logits
